# revision 2
# baseline (speedup 1.0000x reference)
"""Trainium2 Bass kernel for nn_DirectModel_58909771432127 (retrieval_knn).

Computes per-sample argmin over 10 templates of MSE distance:
    out[b] = argmin_t mean((x[b] - templates[t])**2)

Math: argmin_t ||x-t||^2 = argmax_t (x . t - 0.5*||t||^2)   (x_sq is constant
per sample, so it drops out of the argmin).

Per core (pure data parallel over batch, 262144/8 = 32768 samples):
  - for each 128-sample tile, PE-transpose X [128, 784] into 7 chunks
    xT[c] = [112, 512] (4 tiles grouped) so pixels land on partitions,
  - 7 accumulating fp32 matmuls with xT chunk as stationary and the tiny
    transposed-template chunk [112, 10] as moving operand -> PSUM scores
    [128 samples, 10 templates],
  - DVE adds the -0.5*t_sq bias (replicated [128, 10]) while copying PSUM
    -> SBUF, then max / max_index give the argmax index per sample,
  - indices accumulate into a [128, 256] SBUF tile (one column per sample
    tile); two final PE transposes turn that into two contiguous 64KB
    DMA-out blocks.
"""
import numpy as np

import concourse.bacc as bacc
import concourse.mybir as mybir
import concourse.tile as tile
from concourse.bass_utils import run_bass_kernel_spmd

F32 = mybir.dt.float32
U32 = mybir.dt.uint32
AF = mybir.ActivationFunctionType

N_CORES = 8
B = 262144
S = B // N_CORES          # samples per core = 32768
P = 784                   # pixels
NT = 10                   # templates
KC = 112                  # contraction chunk (784 = 7*112)
NCHUNK = 7
TILES = S // 128          # 256 sample tiles per core
GROUPS = TILES // 4       # 64 groups of 4 tiles (512 samples)

_NC_CACHE = {}


def _build_nc():
    nc = bacc.Bacc("TRN2", target_bir_lowering=False, debug=False)

    x_d = nc.dram_tensor("x", [S, P], F32, kind="ExternalInput")
    t_d = nc.dram_tensor("t", [NT, P], F32, kind="ExternalInput")
    o_d = nc.dram_tensor("o", [TILES, 128], F32, kind="ExternalOutput")

    with tile.TileContext(nc) as tc:
        with (
            tc.tile_pool(name="const", bufs=1) as cpool,
            tc.tile_pool(name="xin", bufs=8) as xpool,
            tc.tile_pool(name="xt", bufs=14) as xtpool,
            tc.tile_pool(name="small", bufs=3) as spool,
            tc.tile_pool(name="pst", bufs=3, space="PSUM") as pst,
            tc.tile_pool(name="scps", bufs=4, space="PSUM") as scps,
            tc.tile_pool(name="startps", bufs=1, space="PSUM") as startps,
        ):
            # ---------------- one-time setup ----------------
            # identity matrix via iota + is_equal
            ident = cpool.tile([128, 128], F32)
            rowi = cpool.tile([128, 1], F32)
            nc.gpsimd.iota(rowi[:], [[0, 1]], base=0, channel_multiplier=1,
                           allow_small_or_imprecise_dtypes=True)
            coli = cpool.tile([128, 128], F32)
            nc.gpsimd.iota(coli[:], [[1, 128]], base=0, channel_multiplier=0,
                           allow_small_or_imprecise_dtypes=True)
            nc.vector.tensor_scalar(ident[:], coli[:], rowi[:, 0:1], None,
                                    op0=mybir.AluOpType.is_equal)

            # templates [10, 784]
            t_sb = cpool.tile([NT, P], F32)
            nc.sync.dma_start(t_sb[:], t_d.ap())

            # t_sq [10, 1] = sum(t^2) along free dim
            sq_trash = cpool.tile([NT, P], F32)
            t_sq = cpool.tile([NT, 1], F32)
            nc.scalar.activation(sq_trash[:], t_sb[:], AF.Square,
                                 accum_out=t_sq[:])

            # diag10 [10, 10] = diag(t_sq)
            diag10 = cpool.tile([NT, NT], F32)
            nc.vector.tensor_scalar(diag10[:], ident[0:NT, 0:NT], t_sq[:, 0:1],
                                    None, op0=mybir.AluOpType.mult)

            # bias_rep [128, 10] = -0.5 * t_sq broadcast over partitions:
            # ones[10,128].T @ diag10 -> [128, 10], then copy with scale -0.5
            ones10 = cpool.tile([NT, 128], F32)
            nc.vector.memset(ones10[:], 1.0)
            bias_ps = startps.tile([128, NT], F32, tag="startup")
            nc.tensor.matmul(bias_ps[:], ones10[:], diag10[:],
                             start=True, stop=True)
            bias_rep = cpool.tile([128, NT], F32)
            nc.scalar.activation(bias_rep[:], bias_ps[:], AF.Copy, scale=-0.5)

            # transposed template chunks TT[c] = t_sb[:, c*112:(c+1)*112].T
            TT = []
            for c in range(NCHUNK):
                tps = startps.tile([KC, NT], F32, tag="startup")
                nc.tensor.transpose(tps[:], t_sb[:, c * KC:(c + 1) * KC],
                                    ident[0:NT, 0:NT])
                ttc = cpool.tile([KC, NT], F32, tag=f"tt{c}")
                nc.scalar.activation(ttc[:], tps[:], AF.Copy)
                TT.append(ttc)

            # index accumulator: one column per sample tile
            idx_acc = cpool.tile([128, TILES], F32)

            # ---------------- main loop ----------------
            for g in range(GROUPS):
                xs = []
                for j in range(4):
                    xt_in = xpool.tile([128, P], F32, tag="xin")
                    row = (g * 4 + j) * 128
                    nc.sync.dma_start(xt_in[:], x_d.ap()[row:row + 128, :])
                    xs.append(xt_in)

                xT = []
                for c in range(NCHUNK):
                    psc = pst.tile([KC, 512], F32, tag="pst")
                    for j in range(4):
                        nc.tensor.transpose(
                            psc[:, j * 128:(j + 1) * 128],
                            xs[j][:, c * KC:(c + 1) * KC],
                            ident[:],
                        )
                    xtc = xtpool.tile([KC, 512], F32, tag="xt")
                    # split PSUM->SBUF copies between ACT and DVE
                    if c % 3 == 2:
                        nc.vector.tensor_copy(xtc[:], psc[:])
                    else:
                        nc.scalar.activation(xtc[:], psc[:], AF.Copy)
                    xT.append(xtc)

                for j in range(4):
                    scp = scps.tile([128, NT], F32, tag="sc")
                    for c in range(NCHUNK):
                        nc.tensor.matmul(
                            scp[:],
                            xT[c][:, j * 128:(j + 1) * 128],
                            TT[c][:],
                            start=(c == 0), stop=(c == NCHUNK - 1),
                        )
                    scT = spool.tile([128, NT], F32, tag="sct")
                    nc.vector.tensor_add(scT[:], scp[:], bias_rep[:])
                    mx8 = spool.tile([128, 8], F32, tag="mx8")
                    nc.vector.max(mx8[:], scT[:])
                    ix8 = spool.tile([128, 8], U32, tag="ix8")
                    nc.vector.max_index(ix8[:], mx8[:], scT[:])
                    k = g * 4 + j
                    nc.vector.tensor_copy(idx_acc[:, k:k + 1], ix8[:, 0:1])

            # ---------------- tail: transpose idx_acc and write out ----------------
            for n in range(TILES // 128):
                tp = pst.tile([128, 128], F32, tag="pst")
                nc.tensor.transpose(tp[:], idx_acc[:, n * 128:(n + 1) * 128],
                                    ident[:])
                ob = cpool.tile([128, 128], F32, tag="outsb")
                nc.scalar.activation(ob[:], tp[:], AF.Copy)
                nc.sync.dma_start(o_d.ap()[n * 128:(n + 1) * 128, :], ob[:])

    nc.compile()
    return nc


def kernel(x: np.ndarray, templates: np.ndarray) -> np.ndarray:
    assert x.shape == (B, 28, 28) and templates.shape == (NT, 28, 28)
    if "nc" not in _NC_CACHE:
        _NC_CACHE["nc"] = _build_nc()
    nc = _NC_CACHE["nc"]

    xf = np.ascontiguousarray(np.asarray(x, dtype=np.float32).reshape(B, P))
    tf = np.ascontiguousarray(
        np.asarray(templates, dtype=np.float32).reshape(NT, P))

    in_maps = []
    for i in range(N_CORES):
        in_maps.append({"x": xf[i * S:(i + 1) * S], "t": tf})

    res = run_bass_kernel_spmd(nc, in_maps, core_ids=list(range(N_CORES)))
    out = np.empty((B,), dtype=np.float32)
    for i in range(N_CORES):
        out[i * S:(i + 1) * S] = res.results[i]["o"].reshape(S)
    return out


# revision 3
# speedup vs baseline: 1.3302x; 1.3302x over previous
"""Trainium2 Bass kernel for nn_DirectModel_58909771432127 (retrieval_knn).

Computes per-sample argmin over 10 templates of MSE distance:
    out[b] = argmin_t mean((x[b] - templates[t])**2)

Math: argmin_t ||x-t||^2 = argmax_t (x . t - 0.5*||t||^2)   (x_sq is constant
per sample, so it drops out of the argmin).

Per core (pure data parallel over batch, 262144/8 = 32768 samples):
  - for each 128-sample tile, PE-transpose X [128, 784] into 7 chunks
    xT[c] = [112, 512] (4 tiles grouped) so pixels land on partitions,
  - 7 accumulating fp32 matmuls with xT chunk as stationary and the tiny
    transposed-template chunk [112, 10] as moving operand -> PSUM scores
    [128 samples, 10 templates],
  - DVE adds the -0.5*t_sq bias (replicated [128, 10]) while copying PSUM
    -> SBUF, then max / max_index give the argmax index per sample,
  - indices accumulate into a [128, 256] SBUF tile (one column per sample
    tile); two final PE transposes turn that into two contiguous 64KB
    DMA-out blocks.
"""
import numpy as np

import concourse.bacc as bacc
import concourse.mybir as mybir
import concourse.tile as tile
from concourse.bass_utils import run_bass_kernel_spmd

F32 = mybir.dt.float32
U32 = mybir.dt.uint32
AF = mybir.ActivationFunctionType

N_CORES = 8
B = 262144
S = B // N_CORES          # samples per core = 32768
P = 784                   # pixels
NT = 10                   # templates
KC = 112                  # contraction chunk (784 = 7*112)
NCHUNK = 7
TILES = S // 128          # 256 sample tiles per core
GROUPS = TILES // 4       # 64 groups of 4 tiles (512 samples)

_NC_CACHE = {}


def _build_nc(reps: int = 1):
    nc = bacc.Bacc("TRN2", target_bir_lowering=False, debug=False)

    x_d = nc.dram_tensor("x", [S, P], F32, kind="ExternalInput")
    t_d = nc.dram_tensor("t", [NT, P], F32, kind="ExternalInput")
    o_d = nc.dram_tensor("o", [TILES, 128], F32, kind="ExternalOutput")

    with tile.TileContext(nc) as tc:
        with (
            tc.tile_pool(name="const", bufs=1) as cpool,
            tc.tile_pool(name="xin", bufs=8) as xpool,
            tc.tile_pool(name="xt", bufs=14) as xtpool,
            tc.tile_pool(name="small", bufs=3) as spool,
            tc.tile_pool(name="pst", bufs=3, space="PSUM") as pst,
            tc.tile_pool(name="scps", bufs=4, space="PSUM") as scps,
            tc.tile_pool(name="startps", bufs=1, space="PSUM") as startps,
        ):
            # ---------------- one-time setup ----------------
            # identity matrix via iota + is_equal
            ident = cpool.tile([128, 128], F32)
            rowi = cpool.tile([128, 1], F32)
            nc.gpsimd.iota(rowi[:], [[0, 1]], base=0, channel_multiplier=1,
                           allow_small_or_imprecise_dtypes=True)
            coli = cpool.tile([128, 128], F32)
            nc.gpsimd.iota(coli[:], [[1, 128]], base=0, channel_multiplier=0,
                           allow_small_or_imprecise_dtypes=True)
            nc.vector.tensor_scalar(ident[:], coli[:], rowi[:, 0:1], None,
                                    op0=mybir.AluOpType.is_equal)

            # templates [10, 784]
            t_sb = cpool.tile([NT, P], F32)
            nc.sync.dma_start(t_sb[:], t_d.ap())

            # t_sq [10, 1] = sum(t^2) along free dim
            sq_trash = cpool.tile([NT, P], F32)
            t_sq = cpool.tile([NT, 1], F32)
            nc.scalar.activation(sq_trash[:], t_sb[:], AF.Square,
                                 accum_out=t_sq[:])

            # diag10 [10, 10] = diag(t_sq)
            diag10 = cpool.tile([NT, NT], F32)
            nc.vector.tensor_scalar(diag10[:], ident[0:NT, 0:NT], t_sq[:, 0:1],
                                    None, op0=mybir.AluOpType.mult)

            # bias_rep [128, 10] = -0.5 * t_sq broadcast over partitions:
            # ones[10,128].T @ diag10 -> [128, 10], then copy with scale -0.5
            ones10 = cpool.tile([NT, 128], F32)
            nc.vector.memset(ones10[:], 1.0)
            bias_ps = startps.tile([128, NT], F32, tag="startup")
            nc.tensor.matmul(bias_ps[:], ones10[:], diag10[:],
                             start=True, stop=True)
            bias_rep = cpool.tile([128, NT], F32)
            nc.scalar.activation(bias_rep[:], bias_ps[:], AF.Copy, scale=-0.5)

            # transposed template chunks TT[c] = t_sb[:, c*112:(c+1)*112].T
            TT = []
            for c in range(NCHUNK):
                tps = startps.tile([KC, NT], F32, tag="startup")
                nc.tensor.transpose(tps[:], t_sb[:, c * KC:(c + 1) * KC],
                                    ident[0:NT, 0:NT])
                ttc = cpool.tile([KC, NT], F32, tag=f"tt{c}")
                nc.scalar.activation(ttc[:], tps[:], AF.Copy)
                TT.append(ttc)

            # index accumulator: one column per sample tile
            idx_acc = cpool.tile([128, TILES], F32)

            # ---------------- main loop ----------------
            for _rep in range(reps):
              for g in range(GROUPS):
                xs = []
                for j in range(4):
                    xt_in = xpool.tile([128, P], F32, tag="xin")
                    row = (g * 4 + j) * 128
                    nc.sync.dma_start(xt_in[:], x_d.ap()[row:row + 128, :])
                    xs.append(xt_in)

                xT = []
                for c in range(NCHUNK):
                    psc = pst.tile([KC, 512], F32, tag="pst")
                    for j in range(4):
                        nc.tensor.transpose(
                            psc[:, j * 128:(j + 1) * 128],
                            xs[j][:, c * KC:(c + 1) * KC],
                            ident[:],
                        )
                    xtc = xtpool.tile([KC, 512], F32, tag="xt")
                    # split PSUM->SBUF copies between ACT and DVE
                    if c % 3 == 2:
                        nc.vector.tensor_copy(xtc[:], psc[:])
                    else:
                        nc.scalar.activation(xtc[:], psc[:], AF.Copy)
                    xT.append(xtc)

                for j in range(4):
                    scp = scps.tile([128, NT], F32, tag="sc")
                    for c in range(NCHUNK):
                        nc.tensor.matmul(
                            scp[:],
                            xT[c][:, j * 128:(j + 1) * 128],
                            TT[c][:],
                            start=(c == 0), stop=(c == NCHUNK - 1),
                        )
                    scT = spool.tile([128, NT], F32, tag="sct")
                    nc.vector.tensor_add(scT[:], scp[:], bias_rep[:])
                    mx8 = spool.tile([128, 8], F32, tag="mx8")
                    nc.vector.max(mx8[:], scT[:])
                    ix8 = spool.tile([128, 8], U32, tag="ix8")
                    nc.vector.max_index(ix8[:], mx8[:], scT[:])
                    k = g * 4 + j
                    nc.vector.tensor_copy(idx_acc[:, k:k + 1], ix8[:, 0:1])

            # ---------------- tail: transpose idx_acc and write out ----------------
            for n in range(TILES // 128):
                tp = pst.tile([128, 128], F32, tag="pst")
                nc.tensor.transpose(tp[:], idx_acc[:, n * 128:(n + 1) * 128],
                                    ident[:])
                ob = cpool.tile([128, 128], F32, tag="outsb")
                nc.scalar.activation(ob[:], tp[:], AF.Copy)
                nc.sync.dma_start(o_d.ap()[n * 128:(n + 1) * 128, :], ob[:])

    nc.compile()
    return nc


def kernel(x: np.ndarray, templates: np.ndarray) -> np.ndarray:
    assert x.shape == (B, 28, 28) and templates.shape == (NT, 28, 28)
    if "nc" not in _NC_CACHE:
        _NC_CACHE["nc"] = _build_nc()
    nc = _NC_CACHE["nc"]

    xf = np.ascontiguousarray(np.asarray(x, dtype=np.float32).reshape(B, P))
    tf = np.ascontiguousarray(
        np.asarray(templates, dtype=np.float32).reshape(NT, P))

    in_maps = []
    for i in range(N_CORES):
        in_maps.append({"x": xf[i * S:(i + 1) * S], "t": tf})

    res = run_bass_kernel_spmd(nc, in_maps, core_ids=list(range(N_CORES)))
    out = np.empty((B,), dtype=np.float32)
    for i in range(N_CORES):
        out[i * S:(i + 1) * S] = res.results[i]["o"].reshape(S)
    return out
